# revision 39
# baseline (speedup 1.0000x reference)
"""Trainium2 Bass kernel for an 8-layer Mamba stack (nn_NewMamba).

Sharding: data-parallel over batch (16 -> 8 cores x 2).
Layout: activations kept as [channel(partitions), time(free)] per batch elem.
Scan: hardware tensor_tensor_scan (state = dA*state + x) along the free dim,
one full-length recurrence per (i, s) pair. All reductions/accumulations
(y over s, depthwise conv folded into in_proj weights, D-skip, residual) run
on the tensor engine as PSUM-accumulated matmuls, keeping the vector engine
free for scans and the irreducible elementwise products. The per-step
prologue (rmsnorm/in_proj/conv/x_proj/dt) is emitted one SSM-step early so
it overlaps the previous step's scan phase.
"""

import numpy as np

import concourse.bass as bass
import concourse.mybir as mybir
import concourse.tile as tile
from concourse.bass import ds, ts
from concourse.masks import make_identity

FP32 = mybir.dt.float32
BF16 = mybir.dt.bfloat16
AF = mybir.ActivationFunctionType
OP = mybir.AluOpType

H = 256       # hidden
I = 512       # intermediate
S = 16        # ssm state
R = 16        # time step rank
KCONV = 4     # conv kernel
NL = 8        # layers
EPS = 1e-5
B = 16
LFULL = 2048
NCORES = 8
BLOC = B // NCORES   # 2
P = 128
HC = H // P          # 2
ICN = I // P         # 4
OCN = 2 * I // P     # 8
XP80 = 80


def build_program(L=LFULL, n_layers=NL):
    NT = min(512, L)          # matmul free-dim tile
    assert L % P == 0 and L % NT == 0
    NN = L // NT
    nc = bass.Bass()

    # ---- external I/O ----
    x_in = nc.declare_dram_parameter("x", [BLOC, L, H], FP32, isOutput=False)
    norm_w = nc.declare_dram_parameter("norm_w", [NL, H], FP32, isOutput=False)
    in_w = nc.declare_dram_parameter("in_proj_w", [NL, 2 * I, H], FP32, isOutput=False)
    conv_w = nc.declare_dram_parameter("conv_w", [NL, I, KCONV], FP32, isOutput=False)
    conv_b = nc.declare_dram_parameter("conv_b", [NL, I], FP32, isOutput=False)
    xp_w = nc.declare_dram_parameter("x_proj_w", [NL, R + 2 * S, I], FP32, isOutput=False)
    dt_w = nc.declare_dram_parameter("dt_proj_w", [NL, I, R], FP32, isOutput=False)
    dt_b = nc.declare_dram_parameter("dt_proj_b", [NL, I], FP32, isOutput=False)
    A_log = nc.declare_dram_parameter("A_log", [NL, I, S], FP32, isOutput=False)
    D_in = nc.declare_dram_parameter("D", [NL, I], FP32, isOutput=False)
    out_w = nc.declare_dram_parameter("out_proj_w", [NL, H, I], FP32, isOutput=False)
    y_out = nc.declare_dram_parameter("out", [BLOC, L, H], FP32, isOutput=True)

    # ---- dram scratch ----
    w_hsT = nc.dram_tensor("w_hsT_scr", [n_layers, KCONV, HC, P, I], BF16)
    w_gT = nc.dram_tensor("w_gT_scr", [n_layers, HC, P, I], BF16)
    w_outT = nc.dram_tensor("w_outT_scr", [n_layers, ICN, P, H], BF16)
    w_xpT = nc.dram_tensor("w_xpT_scr", [n_layers, ICN, P, XP80], BF16)
    w_dtT = nc.dram_tensor("w_dtT_scr", [n_layers, R + 1, I], BF16)
    r_dram = nc.dram_tensor("r_scr", [BLOC, 1, L], BF16)
    B_dram = nc.dram_tensor("B_scr", [BLOC, S, L], BF16)
    C_dram = nc.dram_tensor("C_scr", [BLOC, S, L], BF16)
    sg_dram = nc.dram_tensor("sg_scr", [BLOC, ICN, P, L], BF16)

    with tile.TileContext(nc) as tc:
        with (
            tc.tile_pool(name="glob", bufs=1) as pg,
            tc.tile_pool(name="prep", bufs=2) as pw,
            tc.tile_pool(name="prepbig", bufs=1) as pwb,
            tc.tile_pool(name="layer", bufs=1) as pl,
            tc.tile_pool(name="lconst", bufs=2) as plc,
            tc.tile_pool(name="actdb", bufs=2) as pd,
            tc.tile_pool(name="trans", bufs=1) as pt,
            tc.tile_pool(name="ssm", bufs=2) as ps,
            tc.tile_pool(name="dapool", bufs=3) as pda,
            tc.tile_pool(name="sgld", bufs=2) as psg,
            tc.tile_pool(name="brep", bufs=2) as pb,
            tc.tile_pool(name="psum", bufs=3, space="PSUM") as pp,
            tc.tile_pool(name="psumy", bufs=1, space="PSUM") as ppy,
            tc.tile_pool(name="psumT", bufs=1, space="PSUM") as ppt,
        ):
            # ---- global constants ----
            ident = pg.tile([P, P], FP32, name="ident")
            make_identity(nc, ident)
            ident_bf = pg.tile([P, P], BF16, name="ident_bf")
            nc.vector.tensor_copy(ident_bf, ident)
            ones_col = pg.tile([P, 1], BF16, name="ones_col")
            nc.vector.memset(ones_col, 1.0)
            eps_col = pg.tile([P, 1], FP32, name="eps_col")
            nc.vector.memset(eps_col, EPS)
            one_col = pg.tile([P, 1], FP32, name="one_col")
            nc.vector.memset(one_col, 1.0)
            em07_col = pg.tile([P, 1], FP32, name="em07_col")
            nc.vector.memset(em07_col, 0.4965853037914095)

            # ---- transpose x into [H, L] bf16 layout, SBUF-resident ----
            xT_res = [
                [pg.tile([P, L], BF16, name=f"xTr{b}_{h}") for h in range(HC)]
                for b in range(BLOC)
            ]
            for b in range(BLOC):
                for tc_i in range(L // P):
                    xt_ld = pwb.tile([P, H], FP32, name="xio_small")
                    nc.sync.dma_start(xt_ld, x_in[b, ts(tc_i, P), :])
                    for hc in range(HC):
                        pst = ppt.tile([P, P], FP32, name="pst")
                        nc.tensor.matmul(pst, xt_ld[:, ts(hc, P)], ident)
                        nc.vector.tensor_copy(xT_res[b][hc][:, ts(tc_i, P)], pst)

            # ---- weight prep (once) ----
            for li in range(n_layers):
                nw_col = [pw.tile([P, 1], FP32, name=f"nw_pre{h}") for h in range(HC)]
                for hc in range(HC):
                    nc.sync.dma_start(nw_col[hc], norm_w[li, ts(hc, P)][:, None])
                # hs half of in_proj fused with depthwise conv:
                # W_k^T[h, c] = in_w[c, h] * conv_w[c, k] * norm_w[h]
                cdiag_p = [pwb.tile([P, P], BF16, name=f"cdp{k}") for k in range(KCONV)]
                for oc in range(ICN):
                    cw_ld = pw.tile([P, KCONV], FP32, name="cw_pre")
                    nc.sync.dma_start(cw_ld, conv_w[li, ts(oc, P), :])
                    for k in range(KCONV):
                        nc.vector.tensor_scalar_mul(
                            cdiag_p[k], ident_bf, cw_ld[:, k : k + 1]
                        )
                    wtile = pw.tile([P, H], BF16, name="w_ldh")
                    wld = pwb.tile([P, H], FP32, name="w_ldh32")
                    nc.sync.dma_start(wld, in_w[li, ts(oc, P), :])
                    nc.vector.tensor_copy(wtile, wld)
                    for k in range(KCONV):
                        for hc in range(HC):
                            pst = ppt.tile([P, P], FP32, name="pst")
                            nc.tensor.matmul(pst, wtile[:, ts(hc, P)], cdiag_p[k])
                            stg = pw.tile([P, P], BF16, name="wkstg")
                            nc.scalar.activation(stg, pst, AF.Copy, scale=nw_col[hc])
                            nc.sync.dma_start(w_hsT[li, k, hc][:, ts(oc, P)], stg)
                # gate half of in_proj (plain transpose, norm_w folded)
                for oc in range(ICN, OCN):
                    wtile = pwb.tile([P, I], FP32, name="w_ld2")[:, :H]
                    nc.sync.dma_start(wtile, in_w[li, ts(oc, P), :])
                    for hc in range(HC):
                        pst = ppt.tile([P, P], FP32, name="pst")
                        nc.tensor.matmul(pst, wtile[:, ts(hc, P)], ident)
                        stg = pw.tile([P, P], BF16, name="wkstg")
                        nc.scalar.activation(stg, pst, AF.Copy, scale=nw_col[hc])
                        nc.sync.dma_start(w_gT[li, hc][:, ts(oc - ICN, P)], stg)
                # out_proj: want lhsT [I, H] = out_w.T
                woutT_sb = [pwb.tile([P, H], BF16, name=f"woutT_sb{c}") for c in range(ICN)]
                for hc in range(HC):
                    wtile = pwb.tile([P, I], FP32, name="w_ld2")
                    nc.sync.dma_start(wtile, out_w[li, ts(hc, P), :])
                    for ic in range(ICN):
                        pst = ppt.tile([P, P], FP32, name="pst")
                        nc.tensor.matmul(pst, wtile[:, ts(ic, P)], ident)
                        nc.scalar.copy(woutT_sb[ic][:, ts(hc, P)], pst)
                for ic in range(ICN):
                    nc.sync.dma_start(w_outT[li, ic], woutT_sb[ic])
                # x_proj: want lhsT [I, 48] = xp_w.T (padded layout 0:16,32:48,64:80)
                xp_sb = pwb.tile([R + 2 * S, I], FP32, name="w_ld2")
                nc.sync.dma_start(xp_sb, xp_w[li])
                for ic in range(ICN):
                    pst = ppt.tile([P, P], FP32, name="pst")
                    nc.tensor.matmul(
                        pst[:, : R + 2 * S], xp_sb[:, ts(ic, P)],
                        ident[: R + 2 * S, : R + 2 * S],
                    )
                    wx = pw.tile([P, XP80], BF16, name="wx")
                    nc.vector.memset(wx, 0.0)
                    nc.scalar.copy(wx[:, :R], pst[:, :R])        # dt rows 0:16
                    nc.scalar.copy(wx[:, 32:48], pst[:, R : R + S])       # B -> 32:48
                    nc.scalar.copy(wx[:, 64:80], pst[:, R + S : R + 2 * S])  # C -> 64:80
                    nc.sync.dma_start(w_xpT[li, ic], wx)
                # dt_proj: want lhsT [R+1, I]: rows 0..R-1 = dt_w.T, row R = dt_b
                wdt32 = pwb.tile([R + 1, I], FP32, name="wdt32")
                for ic in range(ICN):
                    wtile = pwb.tile([P, R], FP32, name="w_ld3")
                    nc.sync.dma_start(wtile, dt_w[li, ts(ic, P), :])
                    pst = ppt.tile([P, P], FP32, name="pst")
                    nc.tensor.matmul(pst[:R], wtile, ident)
                    nc.scalar.copy(wdt32[:R, ts(ic, P)], pst[:R])
                nc.sync.dma_start(wdt32[R : R + 1, :], dt_b[li][None, :])
                wdt_sb = pwb.tile([R + 1, I], BF16, name="wdt_sb")
                nc.vector.tensor_copy(wdt_sb, wdt32)
                nc.sync.dma_start(w_dtT[li], wdt_sb)

            # ================= layers (software-pipelined emission) =========
            def load_weights(li):
                w = {}
                w["hs"] = [
                    [pl.tile([P, I], BF16, name=f"w_hs{h}_{k}") for k in range(KCONV)]
                    for h in range(HC)
                ]
                w["g"] = [pl.tile([P, I], BF16, name=f"w_g{h}") for h in range(HC)]
                for hc in range(HC):
                    for k in range(KCONV):
                        nc.sync.dma_start(w["hs"][hc][k], w_hsT[li, k, hc])
                    nc.sync.dma_start(w["g"][hc], w_gT[li, hc])
                w["out"] = [plc.tile([P, H], BF16, name=f"w_out{c}") for c in range(ICN)]
                w["xp"] = [pl.tile([P, XP80], BF16, name=f"w_xp{c}") for c in range(ICN)]
                for ic in range(ICN):
                    nc.sync.dma_start(w["out"][ic], w_outT[li, ic])
                    nc.sync.dma_start(w["xp"][ic], w_xpT[li, ic])
                w["dt"] = pl.tile([R + 1, I], BF16, name="w_dt")
                nc.sync.dma_start(w["dt"], w_dtT[li])
                # ssm-phase constants (alive through this layer's scans -> db pool)
                w["cb"] = [plc.tile([P, 1], FP32, name=f"cb{c}") for c in range(ICN)]
                w["a_neg"] = [plc.tile([P, S], FP32, name=f"an{c}") for c in range(ICN)]
                w["a_bias"] = [plc.tile([P, S], FP32, name=f"ab{c}") for c in range(ICN)]
                w["ddiag"] = [plc.tile([P, P], BF16, name=f"dd{c}") for c in range(ICN)]
                for ic in range(ICN):
                    d_ld = pt.tile([P, 1], FP32, name="d_ld")
                    nc.sync.dma_start(d_ld, D_in[li, ts(ic, P)][:, None])
                    nc.scalar.activation(w["ddiag"][ic], ident_bf, AF.Copy, scale=d_ld)
                    nc.sync.dma_start(w["cb"][ic], conv_b[li, ts(ic, P)][:, None])
                    atile = pt.tile([P, S], FP32, name="a_ld")
                    nc.sync.dma_start(atile, A_log[li, ts(ic, P), :])
                    nc.scalar.activation(w["a_neg"][ic], atile, AF.Exp)
                    nc.vector.tensor_scalar_mul(w["a_neg"][ic], w["a_neg"][ic], -1.0)
                    nc.vector.tensor_scalar_mul(w["a_bias"][ic], w["a_neg"][ic], 0.7)
                return w

            def prologue(li, b, w):
                st = {}
                # residual stream lives in persistent SBUF tiles
                xT = xT_res[b]
                st["xT"] = xT
                # rmsnorm (norm_w folded into in_proj weights)
                hn_pad = [pt.tile([P, KCONV + L], BF16, name=f"hn{h}") for h in range(HC)]
                hn = [hn_pad[h][:, KCONV:] for h in range(HC)]
                for hc in range(HC):
                    nc.gpsimd.memset(hn_pad[hc][:, 0:KCONV], 0.0)
                    nc.scalar.activation(hn[hc], xT[hc], AF.Square)
                for nn in range(NN):
                    ms_ps = pp.tile([P, NT], FP32, name="psm")[:1]
                    for hc in range(HC):
                        nc.tensor.matmul(
                            ms_ps, ones_col, hn[hc][:, ts(nn, NT)],
                            start=(hc == 0), stop=(hc == HC - 1),
                        )
                    # r = (mean_sq + eps)^-0.5 = exp(-0.5*ln(ms/H + eps))
                    nc.scalar.activation(
                        ms_ps, ms_ps, AF.Ln, bias=eps_col[:1], scale=1.0 / H
                    )
                    r16 = pt.tile([1, NT], BF16, name="r16")
                    nc.scalar.activation(r16, ms_ps, AF.Exp, scale=-0.5)
                    nc.sync.dma_start(r_dram[b][:, ts(nn, NT)], r16)
                r_rep = pt.tile([P, L], BF16, name="r_rep")
                nc.sync.dma_start(r_rep, r_dram[b].to_broadcast((P, L)))
                for hc in range(HC):
                    nc.scalar.copy(hn[hc], xT[hc])
                    nc.vector.tensor_tensor(hn[hc], hn[hc], r_rep, op=OP.mult)

                # in_proj + depthwise causal conv (fused on PE) + silu -> u
                u = [pl.tile([P, L], BF16, name=f"u{c}") for c in range(ICN)]
                for ic in range(ICN):
                    for nn in range(NN):
                        pcv = pp.tile([P, NT], FP32, name="psm")
                        for k in range(KCONV):
                            for hc in range(HC):
                                nc.tensor.matmul(
                                    pcv,
                                    w["hs"][hc][k][:, ts(ic, P)],
                                    hn_pad[hc][:, 1 + k + nn * NT : 1 + k + nn * NT + NT],
                                    start=(k == 0 and hc == 0),
                                    stop=(k == KCONV - 1 and hc == HC - 1),
                                )
                        nc.scalar.activation(
                            u[ic][:, ts(nn, NT)], pcv, AF.Silu, bias=w["cb"][ic]
                        )
                st["u"] = u
                # gate half of in_proj, silu'd and spilled to dram
                for ic in range(ICN):
                    for nn in range(NN):
                        psm = pp.tile([P, NT], FP32, name="psm")
                        for hc in range(HC):
                            nc.tensor.matmul(
                                psm, w["g"][hc][:, ts(ic, P)], hn[hc][:, ts(nn, NT)],
                                start=(hc == 0), stop=(hc == HC - 1),
                            )
                        sgt = pt.tile([P, NT], BF16, name="sgt")
                        nc.scalar.activation(sgt, psm, AF.Silu)
                        nc.sync.dma_start(sg_dram[b, ic][:, ts(nn, NT)], sgt)

                # x_proj (fused with dt_proj) -> dtp, B, C
                dtp = [pd.tile([P, L], BF16, name=f"dtp{c}") for c in range(ICN)]
                for nn in range(NN):
                    ps48_f = pp.tile([P, NT], FP32, name="psm")
                    ps48 = ps48_f[:XP80]
                    for ic in range(ICN):
                        nc.tensor.matmul(
                            ps48, w["xp"][ic], u[ic][:, ts(nn, NT)],
                            start=(ic == 0), stop=(ic == ICN - 1),
                        )
                    dtr_nn = psg.tile([R + 1, NT], BF16, name="dtr_nn")
                    nc.vector.memset(dtr_nn, 1.0)  # row R = ones (bias row)
                    nc.scalar.copy(dtr_nn[0:R], ps48[0:R])
                    bt = pt.tile([S, NT], BF16, name="bt")
                    nc.scalar.copy(bt, ps48[32:48])
                    nc.sync.dma_start(B_dram[b][:, ts(nn, NT)], bt)
                    ct = pt.tile([S, NT], BF16, name="bt")
                    nc.scalar.copy(ct, ps48[64:80])
                    nc.sync.dma_start(C_dram[b][:, ts(nn, NT)], ct)
                    for mc in range(ICN):
                        psd = pp.tile([P, NT], FP32, name="psm")
                        nc.tensor.matmul(psd, w["dt"][:, ts(mc, P)], dtr_nn)
                        nc.scalar.activation(psd, psd, AF.Exp)
                        # softplus(x)-0.7 = ln(e^-0.7*exp(x) + e^-0.7)
                        nc.scalar.activation(
                            dtp[mc][:, ts(nn, NT)], psd, AF.Ln,
                            bias=em07_col, scale=0.4965853037914095,
                        )
                st["dtp"] = dtp
                # dtu = dt * u
                dtu = [pd.tile([P, L], BF16, name=f"dtu{c}") for c in range(ICN)]
                for mc in range(ICN):
                    nc.vector.scalar_tensor_tensor(
                        dtu[mc], dtp[mc], 0.7, u[mc], op0=OP.add, op1=OP.mult
                    )
                st["dtu"] = dtu
                return st

            def ssm(li, b, w, st):
                y_ssm = [pl.tile([P, L], BF16, name=f"yss{c}") for c in range(ICN)]
                for ic in range(ICN):
                    yps = ppy.tile([P, L], FP32, name="yps")
                    for nn in range(NN):
                        # D-skip term starts the accumulation
                        nc.tensor.matmul(
                            yps[:, ts(nn, NT)], w["ddiag"][ic],
                            st["u"][ic][:, ts(nn, NT)],
                            start=True, stop=False,
                        )
                    for s in range(S):
                        B_rep = pb.tile([P, L], BF16, name="B_rep")
                        nc.sync.dma_start(
                            B_rep, B_dram[b][s : s + 1, :].to_broadcast((P, L))
                        )
                        C_rep = pb.tile([P, L], BF16, name="C_rep")
                        nc.sync.dma_start(
                            C_rep, C_dram[b][s : s + 1, :].to_broadcast((P, L))
                        )
                        dA = pda.tile([P, L], BF16, name="dA")
                        nc.scalar.activation(
                            dA, st["dtp"][ic], AF.Exp,
                            bias=w["a_bias"][ic][:, s : s + 1],
                            scale=w["a_neg"][ic][:, s : s + 1],
                        )
                        xt = ps.tile([P, L], BF16, name="xt")
                        nc.vector.tensor_tensor(xt, st["dtu"][ic], B_rep, op=OP.mult)
                        hscan = ps.tile([P, L], BF16, name="hscan")
                        nc.vector.tensor_tensor_scan(
                            hscan, dA, xt, 0.0, op0=OP.mult, op1=OP.add
                        )
                        hC = ps.tile([P, L], BF16, name="xt")
                        nc.vector.tensor_tensor(hC, hscan, C_rep, op=OP.mult)
                        for nn in range(NN):
                            nc.tensor.matmul(
                                yps[:, ts(nn, NT)], ident_bf, hC[:, ts(nn, NT)],
                                start=False, stop=(s == S - 1),
                            )
                    # gate: y_ssm = ypsum * silu(gate), gate reloaded from dram
                    for nn in range(NN):
                        sgl = psg.tile([P, NT], BF16, name="sgl")
                        nc.sync.dma_start(sgl, sg_dram[b, ic][:, ts(nn, NT)])
                        nc.vector.tensor_tensor(
                            y_ssm[ic][:, ts(nn, NT)],
                            yps[:, ts(nn, NT)], sgl, op=OP.mult,
                        )
                st["y"] = y_ssm

            def outproj(li, b, w, st):
                xT = st["xT"]
                for hc in range(HC):
                    for nn in range(NN):
                        pso = pp.tile([P, NT], FP32, name="psm")
                        nc.tensor.matmul(
                            pso, ident_bf, xT[hc][:, ts(nn, NT)],
                            start=True, stop=False,
                        )
                        for ic in range(ICN):
                            nc.tensor.matmul(
                                pso, w["out"][ic][:, ts(hc, P)],
                                st["y"][ic][:, ts(nn, NT)],
                                start=False, stop=(ic == ICN - 1),
                            )
                        nc.scalar.copy(xT[hc][:, ts(nn, NT)], pso)

            # pipelined emission: prologue of step k+1 is emitted before the
            # ssm of step k, so its act/pe legs overlap the scan phase.
            steps = [(li, b) for li in range(n_layers) for b in range(BLOC)]
            weights = {}
            states = {}
            weights[0] = load_weights(0)
            states[0] = prologue(0, 0, weights[0])
            for idx, (li, b) in enumerate(steps):
                nidx = idx + 1
                if nidx < len(steps):
                    nli, nb = steps[nidx]
                    if nli != li:
                        weights[nli] = load_weights(nli)
                    states[nidx] = prologue(nli, nb, weights[nli])
                ssm(li, b, weights[li], states[idx])
                outproj(li, b, weights[li], states[idx])
                del states[idx]

            # ---- transpose x back to [L, H] and write out ----
            for b in range(BLOC):
                for tc_i in range(L // P):
                    o_sb = pwb.tile([P, H], FP32, name="xio_small")
                    for hc in range(HC):
                        pst = ppt.tile([P, P], FP32, name="pst")
                        nc.tensor.matmul(
                            pst, xT_res[b][hc][:, ts(tc_i, P)], ident_bf
                        )
                        nc.vector.tensor_copy(o_sb[:, ts(hc, P)], pst)
                    nc.sync.dma_start(y_out[b, ts(tc_i, P), :], o_sb)

    return nc


def _split_matmul_waits(nc):
    """walrus codegen allows limited sync waits per instruction;
    hoist extras into EventSemaphore instructions on the same engine."""
    ctr = 0
    for fn in nc.m.functions:
        for bb in fn.blocks:
            insts = bb.instructions
            out = []
            changed = False
            for inst in insts:
                si = inst.sync_info
                if (
                    not isinstance(inst, mybir.InstEventSemaphore)
                    and si is not None
                    and si.on_wait
                    and len(si.on_wait) > 1
                ):
                    waits = list(si.on_wait)
                    for w in waits[: -1]:
                        ev = mybir.InstEventSemaphore(
                            name=f"I-mmwait-{ctr}",
                            engine=inst.engine,
                            sync_info=mybir.SyncInfo(on_wait=[w], on_update=[]),
                            ins=[],
                            outs=[],
                        )
                        ctr += 1
                        out.append(ev)
                    inst.sync_info = mybir.SyncInfo(
                        on_wait=[waits[-1]], on_update=list(si.on_update or [])
                    )
                    changed = True
                out.append(inst)
            if changed:
                bb.instructions = out
    return nc


def kernel(**inputs):
    from concourse.bass_utils import run_bass_kernel_spmd

    x = np.asarray(inputs["x"], dtype=np.float32)
    Bfull, L, _ = x.shape
    nc = build_program(L=L, n_layers=NL)
    _split_matmul_waits(nc)

    weight_names = [
        "norm_w", "in_proj_w", "conv_w", "conv_b", "x_proj_w",
        "dt_proj_w", "dt_proj_b", "A_log", "D", "out_proj_w",
    ]
    weights = {k: np.asarray(inputs[k], dtype=np.float32) for k in weight_names}

    in_maps = []
    for c in range(NCORES):
        m = {"x": x[c * BLOC : (c + 1) * BLOC]}
        m.update(weights)
        in_maps.append(m)

    res = run_bass_kernel_spmd(nc, in_maps, core_ids=list(range(NCORES)))
    out = np.concatenate([r["out"] for r in res.results], axis=0)
    return out
